# revision 16
# baseline (speedup 1.0000x reference)
"""Bass/Trainium2 kernel for nn_DecoderBlock (masked block-sparse linear +
BatchNorm(train) + Swish), sharded over C_OUT blocks across 8 NeuronCores.

Contract: kernel(**inputs) takes the FULL inputs from setup_inputs() and
returns the FULL [B, C_OUT, F_OUT] output.

Sharding: core k owns output channels [4k, 4k+4). With the reference's
block mask (o//4 == c//4) each core needs only input channels [4k, 4k+4),
so the useful slice of W (1/8 of it) is read from HBM exactly once across
the 8 cores, and every core holds the whole batch for its features =>
BatchNorm statistics are local (no collectives).

Math notes:
 - bias cancels exactly through BatchNorm's mean subtraction -> dropped.
 - single-pass bf16 matmul (fp32 PSUM accumulate): end-to-end rel err vs
   the fp32 reference is ~3.7e-3 (measured), comfortably inside the 2e-2
   gate, at 1/3 the PE time and 1/2 the W DMA of the previous bf16x3.
 - BN eps (1e-5) dropped: var is ~1 +/- 0.3 by construction, the effect
   is ~1e-5 relative -- far below the bf16 noise floor.
 - rstd = 1/sqrt(var) via DVE Newton (seed 1.5-0.5*v, 2 steps, fused
   with scalar_tensor_tensor: 3 DVE ops per step). ScalarE runs ONLY
   Silu -> a single ACT table load, hidden behind the PE stream.
 - output stored bf16 (halves output DMA), widened to fp32 on host.

Layout notes (all chosen so every DMA is one dense 2D block -- one
descriptor per transfer, ~0.6us dispatch each):
 - xh  [P, KT*B]       x^T tiled k-major, per-partition contiguous.
 - wh  [P, PT*KT*128]  W^T in p-tile-major chunks: chunk pt is
                       wh[:, pt*KT*128 : (pt+1)*KT*128], so the W stream
                       arrives tile-by-tile and the PE consumes p-major,
                       finishing output tiles staggered ~0.86us apart ->
                       the stats/newton/silu/out-DMA epilogue pipelines
                       behind the PE with only the last tile in the tail.
 - out [P, PT*B] bf16  silu results, per-partition contiguous.

Perf notes:
 - W chunks alternate sync/scalar HWDGE queues (aggregate HBM ~390GB/s);
   x + gamma/beta + out ride the gpsimd/sync queues.
 - small PE warm-up (dummy matmuls) covers the dispatch->first-chunk
   latency so the PE is at full clock when real data lands.
"""

import os

import numpy as np
import ml_dtypes

B = 256
C_IN, F_IN = 32, 256
C_OUT, F_OUT = 32, 256
KERNEL_SIZE = 4
N_CORES = 8
OC_PER_CORE = C_OUT // N_CORES  # 4 output channels per core
P = 128

N_WARM = int(os.environ.get("KERNEL_WARM", "24"))
TRACE = False  # set True (e.g. from test.py) to capture an NTFF profile
LAST_RESULT = {}  # exec_time_ns etc. from the most recent run

_program_cache = {}


def _build_program(kc):
    """Build the SPMD Bass program for kc active input channels per core."""
    import concourse.bass as bass
    import concourse.tile as tile
    import concourse.mybir as mybir

    K = kc * F_IN  # contraction dim
    KT = K // P  # k-tiles of 128
    PT = (OC_PER_CORE * F_OUT) // P  # output-feature tiles of 128 (=8)
    NP = OC_PER_CORE * F_OUT  # per-core output features (=1024)
    WCH = KT * P  # W columns per p-tile chunk
    f32 = mybir.dt.float32
    bf16 = mybir.dt.bfloat16
    AFT = mybir.ActivationFunctionType
    OP = mybir.AluOpType

    nc = bass.Bass()
    xh_d = nc.declare_dram_parameter("xh", [P, KT * B], bf16, isOutput=False)
    wh_d = nc.declare_dram_parameter("wh", [P, PT * WCH], bf16, isOutput=False)
    gb_d = nc.declare_dram_parameter("gb", [P, 3 * PT], f32, isOutput=False)
    out_d = nc.declare_dram_parameter("out", [P, PT * B], bf16, isOutput=True)

    with tile.TileContext(nc) as tc:
        with (
            tc.tile_pool(name="wpool", bufs=1) as wpool,
            tc.tile_pool(name="xpool", bufs=1) as xpool,
            tc.tile_pool(name="spool", bufs=1) as spool,
            tc.tile_pool(name="stat", bufs=1) as stat,
            tc.tile_pool(name="opool", bufs=1) as opool,
            tc.tile_pool(name="psum", bufs=1, space="PSUM") as psum,
        ):
            # --- input DMAs. Priority order matters: each HWDGE queue is
            # FIFO and the queues round-robin for HBM, so x (needed in
            # full by the first output tile) leads both queues, then W
            # chunks interleave sync/scalar in PE consumption order.
            # A tiny leading DMA on each queue absorbs the ~1.5us
            # cold-start so the x chunks flow sooner.
            # warm-up constants first on their engines so the PE warm-up
            # is gated only on a ~100ns memset, not on any DMA.
            warm_w = spool.tile([P, 64], f32, name="warm_w")
            nc.vector.memset(warm_w, 0.0)
            expn = stat.tile([P, PT], f32, name="expn")
            nc.gpsimd.memset(expn, -0.5)
            gb_t = spool.tile([P, 3 * PT], f32, name="gb")
            nc.gpsimd.dma_start(out=gb_t, in_=gb_d.ap())

            xh_all = xpool.tile([P, KT * B], bf16, name="xh_all")
            wh_all = wpool.tile([P, PT * WCH], bf16, name="wh_all")
            XC = 2 * B  # x chunk: 2 k-tiles

            def dma_x(q, c):
                q.dma_start(
                    out=xh_all[:, c * XC : (c + 1) * XC],
                    in_=xh_d.ap()[:, c * XC : (c + 1) * XC],
                )

            def dma_w(q, pt):
                q.dma_start(
                    out=wh_all[:, pt * WCH : (pt + 1) * WCH],
                    in_=wh_d.ap()[:, pt * WCH : (pt + 1) * WCH],
                )

            dma_x(nc.sync, 0)
            dma_x(nc.scalar, 2)
            dma_w(nc.sync, 0)
            dma_w(nc.scalar, 1)
            dma_x(nc.sync, 1)
            dma_x(nc.scalar, 3)
            for pt in range(2, PT):
                dma_w(nc.sync if pt % 2 == 0 else nc.scalar, pt)

            # the whole PSUM as one tile, one bank per output tile:
            # adjacent accumulation groups never contend on a bank, and
            # pair-wise views (bn_stats over two tiles in one call) work.
            ps_all = psum.tile([P, PT, 512], f32, name="ps_all")

            # PE warm-up: dummy matmuls bridge the gap from user-code
            # start (~7.6us, after the framework preamble) to first
            # data (~10.5us), keeping the PE clock ramping so real
            # matmuls issue at the fastest DVFS tier sooner. Each group
            # is closed (start+stop) before tile 0's group re-starts the
            # bank.
            for _ in range(N_WARM):
                nc.tensor.matmul(
                    ps_all[0:16, 0, 0:64],
                    lhsT=warm_w[:, 0:16],
                    rhs=warm_w[:, 0:64],
                    start=True,
                    stop=True,
                )

            stats_p = stat.tile([P, PT, 6], f32, name="stats_p")
            mv_all = stat.tile([P, PT, 2], f32, name="mv_all")
            r_all = stat.tile([P, PT], f32, name="r_all")
            a_all = stat.tile([P, PT], f32, name="a_all")
            c_all = stat.tile([P, PT], f32, name="c_all")
            o_all = opool.tile([P, PT * B], bf16, name="o_all")

            def stats(h0, h1):
                for h in range(h0, h1):
                    nc.vector.bn_stats(out=stats_p[:, h, :], in_=ps_all[:, h, 0:B])
                    nc.vector.bn_aggr(out=mv_all[:, h, :], in_=stats_p[:, h, :])

            def ac_group(h0, h1):
                """a = gamma*rsqrt(var), c = beta - mean*a for tiles
                [h0, h1). rstd = var**-0.5 is ONE GpSimd pow op (the only
                fp tensor_tensor op that engine supports, and it's
                otherwise idle); the DVE adds just 3 small ops per group
                on top of its bn_stats/bn_aggr load."""
                t = stat.tile([P, h1 - h0], f32, name=f"t{h0}")
                nc.gpsimd.tensor_tensor(
                    out=r_all[:, h0:h1],
                    in0=mv_all[:, h0:h1, 1],
                    in1=expn[:, h0:h1],
                    op=OP.pow,
                )
                nc.vector.tensor_mul(
                    out=a_all[:, h0:h1], in0=r_all[:, h0:h1], in1=gb_t[:, h0:h1]
                )
                nc.vector.tensor_mul(
                    out=t, in0=mv_all[:, h0:h1, 0], in1=a_all[:, h0:h1]
                )
                nc.vector.tensor_sub(
                    out=c_all[:, h0:h1], in0=gb_t[:, PT + h0 : PT + h1], in1=t
                )

            na_all = stat.tile([P, PT], f32, name="na_all")

            def ac_single(h):
                """Same as ac_group for ONE tile, but entirely on GpSimd
                via AP-scalar tensor_scalar (c = mean*(-gamma*rstd) + beta
                using the host-precomputed -gamma section of gb). Used for
                the last two tiles: one less cross-engine hop in the tail
                and the DVE is left free to drain its stats backlog."""
                gp = nc.gpsimd
                gp.tensor_tensor(
                    out=r_all[:, h : h + 1],
                    in0=mv_all[:, h : h + 1, 1],
                    in1=expn[:, h : h + 1],
                    op=OP.pow,
                )
                gp.tensor_scalar(
                    a_all[:, h : h + 1], r_all[:, h : h + 1],
                    gb_t[:, h : h + 1], None, OP.mult,
                )
                gp.tensor_scalar(
                    na_all[:, h : h + 1], r_all[:, h : h + 1],
                    gb_t[:, 2 * PT + h : 2 * PT + h + 1], None, OP.mult,
                )
                gp.tensor_scalar(
                    c_all[:, h : h + 1], mv_all[:, h : h + 1, 0],
                    na_all[:, h : h + 1], gb_t[:, PT + h : PT + h + 1],
                    OP.mult, OP.add,
                )

            def silu(h):
                nc.scalar.activation(
                    out=o_all[:, h * B : (h + 1) * B],
                    in_=ps_all[:, h, 0:B],
                    func=AFT.Silu,
                    bias=c_all[:, h : h + 1],
                    scale=a_all[:, h : h + 1],
                )

            def dma_out(h0, h1):
                nc.sync.dma_start(
                    out=out_d.ap()[:, h0 * B : h1 * B],
                    in_=o_all[:, h0 * B : h1 * B],
                )

            # p-major main loop: tile pt's matmuls ride chunk pt of the W
            # stream. Epilogue pipelines behind the PE: paired bn_stats
            # (DVE), rstd per pair via GpSimd pow, a/c on DVE, silu per
            # tile (ACT), out DMA on the sync queue (free after the input
            # dispatches). The last two tiles run their chains singly so
            # only tile 7's chain sits in the tail.
            for pt in range(PT):
                for kt in range(KT):
                    nc.tensor.matmul(
                        ps_all[:, pt, 0:B],
                        lhsT=wh_all[:, pt * WCH + kt * P : pt * WCH + (kt + 1) * P],
                        rhs=xh_all[:, kt * B : (kt + 1) * B],
                        start=kt == 0,
                        stop=kt == KT - 1,
                    )
                if pt in (1, 3, 5):
                    stats(pt - 1, pt + 1)
                    ac_group(pt - 1, pt + 1)
                    silu(pt - 1)
                    silu(pt)
                    dma_out(pt - 1, pt + 1)
                elif pt >= 6:
                    stats(pt, pt + 1)
                    ac_single(pt)
                    silu(pt)
                    dma_out(pt, pt + 1)

    _split_excess_waits(nc)
    return nc


def _split_excess_waits(nc, limit=1):
    """Walrus codegen rejects instructions carrying more than one sync wait;
    hoist excess waits onto same-engine NOPs inserted immediately before."""
    import concourse.mybir as mybir

    for fn in nc.m.functions:
        for blk in fn.blocks:
            new_insts = []
            for inst in blk.instructions:
                si = inst.sync_info
                waits = list(si.on_wait) if (si and si.on_wait) else []
                if len(waits) > limit:
                    extra = waits[:-limit]
                    inst.sync_info.on_wait = waits[-limit:]
                    while extra:
                        chunk, extra = extra[:limit], extra[limit:]
                        nop = mybir.InstNoOp(
                            name=nc.get_next_instruction_name(),
                            engine=inst.engine,
                            ins=[],
                            outs=[],
                            sync_info=mybir.SyncInfo(on_wait=chunk, on_update=[]),
                        )
                        new_insts.append(nop)
                new_insts.append(inst)
            blk.instructions[:] = new_insts


def kernel(x, W, bias, gamma, beta, mask):
    from concourse.bass_utils import run_bass_kernel_spmd

    x = np.asarray(x, dtype=np.float32)
    W = np.asarray(W, dtype=np.float32)
    gamma = np.asarray(gamma, dtype=np.float32)
    beta = np.asarray(beta, dtype=np.float32)
    mask_np = np.asarray(mask).astype(bool)

    groups = [
        list(range(OC_PER_CORE * k, OC_PER_CORE * (k + 1))) for k in range(N_CORES)
    ]
    active = [np.where(mask_np[g].any(axis=0))[0] for g in groups]
    kc = max(1, max(len(a) for a in active))

    if kc not in _program_cache:
        _program_cache[kc] = _build_program(kc)
    nc = _program_cache[kc]

    K = kc * F_IN
    KT = K // P
    PT = (OC_PER_CORE * F_OUT) // P
    NP = OC_PER_CORE * F_OUT

    gamma2 = gamma.reshape(C_OUT, F_OUT)
    beta2 = beta.reshape(C_OUT, F_OUT)
    bf = ml_dtypes.bfloat16

    in_maps = []
    for k in range(N_CORES):
        g = groups[k]
        a = active[k]
        w_eff = np.zeros((OC_PER_CORE, kc, F_OUT, F_IN), dtype=np.float32)
        if len(a):
            w_eff[:, : len(a)] = W[g][:, a] * mask_np[g][:, a][:, :, None, None]
        # wT[k=(j,i), n=(o_local,f)] -> [P, PT, KT, 128] p-tile-major chunks
        wT = w_eff.transpose(1, 3, 0, 2).reshape(K, NP).astype(bf)
        wh = np.ascontiguousarray(
            wT.reshape(KT, P, PT, P).transpose(1, 2, 0, 3)
        ).reshape(P, PT * KT * P)
        xb = np.zeros((B, kc, F_IN), dtype=np.float32)
        if len(a):
            xb[:, : len(a)] = x[:, a, :]
        xT = xb.transpose(1, 2, 0).reshape(K, B).astype(bf)
        xh = np.ascontiguousarray(xT.reshape(KT, P, B).transpose(1, 0, 2)).reshape(
            P, KT * B
        )

        gs = gamma2[g].reshape(NP).reshape(PT, P).T  # [P, PT]
        bs = beta2[g].reshape(NP).reshape(PT, P).T
        gb = np.ascontiguousarray(np.concatenate([gs, bs, -gs], axis=1))

        in_maps.append({"xh": xh, "wh": wh, "gb": gb})

    res = run_bass_kernel_spmd(nc, in_maps, core_ids=list(range(N_CORES)), trace=TRACE)
    LAST_RESULT["exec_time_ns"] = res.exec_time_ns
    LAST_RESULT["mean_exec_time_ns"] = res.mean_exec_time_ns
    LAST_RESULT["trace"] = res.instructions_and_trace

    out = np.empty((B, C_OUT, F_OUT), dtype=np.float32)
    for k in range(N_CORES):
        o = np.asarray(res.results[k]["out"]).astype(np.float32)  # [P, PT*B]
        y = o.reshape(P, PT, B).transpose(1, 0, 2).reshape(NP, B)
        out[:, groups[k], :] = y.T.reshape(B, OC_PER_CORE, F_OUT)
    return out


# revision 17
# speedup vs baseline: 1.1732x; 1.1732x over previous
"""Bass/Trainium2 kernel for nn_DecoderBlock (masked block-sparse linear +
BatchNorm(train) + Swish), sharded over C_OUT blocks across 8 NeuronCores.

Contract: kernel(**inputs) takes the FULL inputs from setup_inputs() and
returns the FULL [B, C_OUT, F_OUT] output.

Sharding: core k owns output channels [4k, 4k+4). With the reference's
block mask (o//4 == c//4) each core needs only input channels [4k, 4k+4),
so the useful slice of W (1/8 of it) is read from HBM exactly once across
the 8 cores, and every core holds the whole batch for its features =>
BatchNorm statistics are local (no collectives).

Math notes:
 - bias cancels exactly through BatchNorm's mean subtraction -> dropped.
 - single-pass bf16 matmul (fp32 PSUM accumulate): end-to-end rel err vs
   the fp32 reference is ~3.7e-3 (measured), comfortably inside the 2e-2
   gate, at 1/3 the PE time and 1/2 the W DMA of the previous bf16x3.
 - BN eps (1e-5) dropped: var is ~1 +/- 0.3 by construction, the effect
   is ~1e-5 relative -- far below the bf16 noise floor.
 - rstd = 1/sqrt(var) via DVE Newton (seed 1.5-0.5*v, 2 steps, fused
   with scalar_tensor_tensor: 3 DVE ops per step). ScalarE runs ONLY
   Silu -> a single ACT table load, hidden behind the PE stream.
 - output stored bf16 (halves output DMA), widened to fp32 on host.

Layout notes (all chosen so every DMA is one dense 2D block -- one
descriptor per transfer, ~0.6us dispatch each):
 - xh  [P, KT*B]       x^T tiled k-major, per-partition contiguous.
 - wh  [P, PT*KT*128]  W^T in p-tile-major chunks: chunk pt is
                       wh[:, pt*KT*128 : (pt+1)*KT*128], so the W stream
                       arrives tile-by-tile and the PE consumes p-major,
                       finishing output tiles staggered ~0.86us apart ->
                       the stats/newton/silu/out-DMA epilogue pipelines
                       behind the PE with only the last tile in the tail.
 - out [P, PT*B] bf16  silu results, per-partition contiguous.

Perf notes:
 - W chunks alternate sync/scalar HWDGE queues (aggregate HBM ~390GB/s);
   x + gamma/beta + out ride the gpsimd/sync queues.
 - small PE warm-up (dummy matmuls) covers the dispatch->first-chunk
   latency so the PE is at full clock when real data lands.
"""

import os

import numpy as np
import ml_dtypes

B = 256
C_IN, F_IN = 32, 256
C_OUT, F_OUT = 32, 256
KERNEL_SIZE = 4
N_CORES = 8
OC_PER_CORE = C_OUT // N_CORES  # 4 output channels per core
P = 128

N_WARM = int(os.environ.get("KERNEL_WARM", "24"))
TRACE = False  # set True (e.g. from test.py) to capture an NTFF profile
LAST_RESULT = {}  # exec_time_ns etc. from the most recent run

_program_cache = {}


def _build_program(kc):
    """Build the SPMD Bass program for kc active input channels per core."""
    import concourse.bass as bass
    import concourse.tile as tile
    import concourse.mybir as mybir

    K = kc * F_IN  # contraction dim
    KT = K // P  # k-tiles of 128
    PT = (OC_PER_CORE * F_OUT) // P  # output-feature tiles of 128 (=8)
    NP = OC_PER_CORE * F_OUT  # per-core output features (=1024)
    WCH = KT * P  # W columns per p-tile chunk
    f32 = mybir.dt.float32
    bf16 = mybir.dt.bfloat16
    AFT = mybir.ActivationFunctionType
    OP = mybir.AluOpType

    nc = bass.Bass()
    xh_d = nc.declare_dram_parameter("xh", [P, KT * B], bf16, isOutput=False)
    wh_d = nc.declare_dram_parameter("wh", [P, PT * WCH], bf16, isOutput=False)
    gb_d = nc.declare_dram_parameter("gb", [P, 3 * PT], f32, isOutput=False)
    out_d = nc.declare_dram_parameter("out", [P, PT * B], bf16, isOutput=True)

    with tile.TileContext(nc) as tc:
        with (
            tc.tile_pool(name="wpool", bufs=1) as wpool,
            tc.tile_pool(name="xpool", bufs=1) as xpool,
            tc.tile_pool(name="spool", bufs=1) as spool,
            tc.tile_pool(name="stat", bufs=1) as stat,
            tc.tile_pool(name="opool", bufs=1) as opool,
            tc.tile_pool(name="psum", bufs=1, space="PSUM") as psum,
        ):
            # --- input DMAs. Priority order matters: each HWDGE queue is
            # FIFO and the queues round-robin for HBM, so x (needed in
            # full by the first output tile) leads both queues, then W
            # chunks interleave sync/scalar in PE consumption order.
            # A tiny leading DMA on each queue absorbs the ~1.5us
            # cold-start so the x chunks flow sooner.
            # warm-up constants first on their engines so the PE warm-up
            # is gated only on a ~100ns memset, not on any DMA.
            warm_w = spool.tile([P, 64], f32, name="warm_w")
            nc.vector.memset(warm_w, 0.0)
            expn = stat.tile([P, PT], f32, name="expn")
            nc.gpsimd.memset(expn, -0.5)
            gb_t = spool.tile([P, 3 * PT], f32, name="gb")
            nc.gpsimd.dma_start(out=gb_t, in_=gb_d.ap())

            xh_all = xpool.tile([P, KT * B], bf16, name="xh_all")
            wh_all = wpool.tile([P, PT * WCH], bf16, name="wh_all")
            XC = 2 * B  # x chunk: 2 k-tiles

            def dma_x(q, c):
                q.dma_start(
                    out=xh_all[:, c * XC : (c + 1) * XC],
                    in_=xh_d.ap()[:, c * XC : (c + 1) * XC],
                )

            def dma_w(q, pt):
                q.dma_start(
                    out=wh_all[:, pt * WCH : (pt + 1) * WCH],
                    in_=wh_d.ap()[:, pt * WCH : (pt + 1) * WCH],
                )

            dma_x(nc.sync, 0)
            dma_x(nc.scalar, 2)
            dma_w(nc.sync, 0)
            dma_w(nc.scalar, 1)
            dma_x(nc.sync, 1)
            dma_x(nc.scalar, 3)
            for pt in range(2, PT):
                dma_w(nc.sync if pt % 2 == 0 else nc.scalar, pt)

            # the whole PSUM as one tile, one bank per output tile:
            # adjacent accumulation groups never contend on a bank, and
            # pair-wise views (bn_stats over two tiles in one call) work.
            ps_all = psum.tile([P, PT, 512], f32, name="ps_all")

            # PE warm-up: dummy matmuls bridge the gap from user-code
            # start (~7.6us, after the framework preamble) to first
            # data (~10.5us), keeping the PE clock ramping so real
            # matmuls issue at the fastest DVFS tier sooner. Each group
            # is closed (start+stop) before tile 0's group re-starts the
            # bank.
            for _ in range(N_WARM):
                nc.tensor.matmul(
                    ps_all[0:16, 0, 0:64],
                    lhsT=warm_w[:, 0:16],
                    rhs=warm_w[:, 0:64],
                    start=True,
                    stop=True,
                )

            stats_p = stat.tile([P, PT, 6], f32, name="stats_p")
            mv_all = stat.tile([P, PT, 2], f32, name="mv_all")
            r_all = stat.tile([P, PT], f32, name="r_all")
            a_all = stat.tile([P, PT], f32, name="a_all")
            c_all = stat.tile([P, PT], f32, name="c_all")
            o_all = opool.tile([P, PT * B], bf16, name="o_all")

            def stats(h0, h1):
                for h in range(h0, h1):
                    nc.vector.bn_stats(out=stats_p[:, h, :], in_=ps_all[:, h, 0:B])
                    nc.vector.bn_aggr(out=mv_all[:, h, :], in_=stats_p[:, h, :])

            def ac_group(h0, h1):
                """a = gamma*rsqrt(var), c = beta - mean*a for tiles
                [h0, h1). rstd = var**-0.5 is ONE GpSimd pow op (the only
                fp tensor_tensor op that engine supports, and it's
                otherwise idle); the DVE adds just 3 small ops per group
                on top of its bn_stats/bn_aggr load."""
                t = stat.tile([P, h1 - h0], f32, name=f"t{h0}")
                nc.gpsimd.tensor_tensor(
                    out=r_all[:, h0:h1],
                    in0=mv_all[:, h0:h1, 1],
                    in1=expn[:, h0:h1],
                    op=OP.pow,
                )
                nc.vector.tensor_mul(
                    out=a_all[:, h0:h1], in0=r_all[:, h0:h1], in1=gb_t[:, h0:h1]
                )
                nc.vector.tensor_mul(
                    out=t, in0=mv_all[:, h0:h1, 0], in1=a_all[:, h0:h1]
                )
                nc.vector.tensor_sub(
                    out=c_all[:, h0:h1], in0=gb_t[:, PT + h0 : PT + h1], in1=t
                )

            na_all = stat.tile([P, PT], f32, name="na_all")

            def ac_single(h):
                """Same as ac_group for ONE tile, but entirely on GpSimd
                via AP-scalar tensor_scalar (c = mean*(-gamma*rstd) + beta
                using the host-precomputed -gamma section of gb). Used for
                the last two tiles: one less cross-engine hop in the tail
                and the DVE is left free to drain its stats backlog."""
                gp = nc.gpsimd
                gp.tensor_tensor(
                    out=r_all[:, h : h + 1],
                    in0=mv_all[:, h : h + 1, 1],
                    in1=expn[:, h : h + 1],
                    op=OP.pow,
                )
                gp.tensor_scalar(
                    a_all[:, h : h + 1], r_all[:, h : h + 1],
                    gb_t[:, h : h + 1], None, OP.mult,
                )
                gp.tensor_scalar(
                    na_all[:, h : h + 1], r_all[:, h : h + 1],
                    gb_t[:, 2 * PT + h : 2 * PT + h + 1], None, OP.mult,
                )
                gp.tensor_scalar(
                    c_all[:, h : h + 1], mv_all[:, h : h + 1, 0],
                    na_all[:, h : h + 1], gb_t[:, PT + h : PT + h + 1],
                    OP.mult, OP.add,
                )

            def silu(h):
                nc.scalar.activation(
                    out=o_all[:, h * B : (h + 1) * B],
                    in_=ps_all[:, h, 0:B],
                    func=AFT.Silu,
                    bias=c_all[:, h : h + 1],
                    scale=a_all[:, h : h + 1],
                )

            def dma_out(h0, h1):
                nc.sync.dma_start(
                    out=out_d.ap()[:, h0 * B : h1 * B],
                    in_=o_all[:, h0 * B : h1 * B],
                )

            # p-major main loop: tile pt's matmuls ride chunk pt of the W
            # stream. Epilogue pipelines behind the PE: paired bn_stats
            # (DVE), rstd per pair via GpSimd pow, a/c on DVE, silu per
            # tile (ACT), out DMA on the sync queue (free after the input
            # dispatches). The last two tiles run their chains singly so
            # only tile 7's chain sits in the tail.
            for pt in range(PT):
                for kt in range(KT):
                    nc.tensor.matmul(
                        ps_all[:, pt, 0:B],
                        lhsT=wh_all[:, pt * WCH + kt * P : pt * WCH + (kt + 1) * P],
                        rhs=xh_all[:, kt * B : (kt + 1) * B],
                        start=kt == 0,
                        stop=kt == KT - 1,
                    )
                if pt in (1, 3, 5):
                    stats(pt - 1, pt + 1)
                    ac_group(pt - 1, pt + 1)
                    silu(pt - 1)
                    silu(pt)
                    dma_out(pt - 1, pt + 1)
                elif pt >= 6:
                    stats(pt, pt + 1)
                    ac_group(pt, pt + 1)
                    silu(pt)
                    dma_out(pt, pt + 1)

    _split_excess_waits(nc)
    return nc


def _split_excess_waits(nc, limit=1):
    """Walrus codegen rejects instructions carrying more than one sync wait;
    hoist excess waits onto same-engine NOPs inserted immediately before."""
    import concourse.mybir as mybir

    for fn in nc.m.functions:
        for blk in fn.blocks:
            new_insts = []
            for inst in blk.instructions:
                si = inst.sync_info
                waits = list(si.on_wait) if (si and si.on_wait) else []
                if len(waits) > limit:
                    extra = waits[:-limit]
                    inst.sync_info.on_wait = waits[-limit:]
                    while extra:
                        chunk, extra = extra[:limit], extra[limit:]
                        nop = mybir.InstNoOp(
                            name=nc.get_next_instruction_name(),
                            engine=inst.engine,
                            ins=[],
                            outs=[],
                            sync_info=mybir.SyncInfo(on_wait=chunk, on_update=[]),
                        )
                        new_insts.append(nop)
                new_insts.append(inst)
            blk.instructions[:] = new_insts


def kernel(x, W, bias, gamma, beta, mask):
    from concourse.bass_utils import run_bass_kernel_spmd

    x = np.asarray(x, dtype=np.float32)
    W = np.asarray(W, dtype=np.float32)
    gamma = np.asarray(gamma, dtype=np.float32)
    beta = np.asarray(beta, dtype=np.float32)
    mask_np = np.asarray(mask).astype(bool)

    groups = [
        list(range(OC_PER_CORE * k, OC_PER_CORE * (k + 1))) for k in range(N_CORES)
    ]
    active = [np.where(mask_np[g].any(axis=0))[0] for g in groups]
    kc = max(1, max(len(a) for a in active))

    if kc not in _program_cache:
        _program_cache[kc] = _build_program(kc)
    nc = _program_cache[kc]

    K = kc * F_IN
    KT = K // P
    PT = (OC_PER_CORE * F_OUT) // P
    NP = OC_PER_CORE * F_OUT

    gamma2 = gamma.reshape(C_OUT, F_OUT)
    beta2 = beta.reshape(C_OUT, F_OUT)
    bf = ml_dtypes.bfloat16

    in_maps = []
    for k in range(N_CORES):
        g = groups[k]
        a = active[k]
        w_eff = np.zeros((OC_PER_CORE, kc, F_OUT, F_IN), dtype=np.float32)
        if len(a):
            w_eff[:, : len(a)] = W[g][:, a] * mask_np[g][:, a][:, :, None, None]
        # wT[k=(j,i), n=(o_local,f)] -> [P, PT, KT, 128] p-tile-major chunks
        wT = w_eff.transpose(1, 3, 0, 2).reshape(K, NP).astype(bf)
        wh = np.ascontiguousarray(
            wT.reshape(KT, P, PT, P).transpose(1, 2, 0, 3)
        ).reshape(P, PT * KT * P)
        xb = np.zeros((B, kc, F_IN), dtype=np.float32)
        if len(a):
            xb[:, : len(a)] = x[:, a, :]
        xT = xb.transpose(1, 2, 0).reshape(K, B).astype(bf)
        xh = np.ascontiguousarray(xT.reshape(KT, P, B).transpose(1, 0, 2)).reshape(
            P, KT * B
        )

        gs = gamma2[g].reshape(NP).reshape(PT, P).T  # [P, PT]
        bs = beta2[g].reshape(NP).reshape(PT, P).T
        gb = np.ascontiguousarray(np.concatenate([gs, bs, -gs], axis=1))

        in_maps.append({"xh": xh, "wh": wh, "gb": gb})

    res = run_bass_kernel_spmd(nc, in_maps, core_ids=list(range(N_CORES)), trace=TRACE)
    LAST_RESULT["exec_time_ns"] = res.exec_time_ns
    LAST_RESULT["mean_exec_time_ns"] = res.mean_exec_time_ns
    LAST_RESULT["trace"] = res.instructions_and_trace

    out = np.empty((B, C_OUT, F_OUT), dtype=np.float32)
    for k in range(N_CORES):
        o = np.asarray(res.results[k]["out"]).astype(np.float32)  # [P, PT*B]
        y = o.reshape(P, PT, B).transpose(1, 0, 2).reshape(NP, B)
        out[:, groups[k], :] = y.T.reshape(B, OC_PER_CORE, F_OUT)
    return out
